# revision 1
# baseline (speedup 1.0000x reference)
"""Trainium2 Bass kernel for nn_Capsule (capsule attention w/ dynamic routing).

Math: in the reference, c = softmax(b, axis=1) is over a size-1 axis, so c == 1
for every routing iteration. The whole module therefore collapses to

    s[b, d] = sum_{j,e} W[0, j, d, e] * x[b, j, e]        (one big matmul)
    out     = squash(s)  -> (B, 1, D)

i.e. a (512, 36*1024) @ (36*1024, 1024) matmul followed by a per-row squash.

Sharding: contraction(K)-parallel over 8 cores. Each core gets K/8 = 4608 rows
of x^T and W^T (laid out host-side in SBUF-friendly [128, kt*free] order, cast
to bf16) and computes a partial (512, 1024) fp32 sum on its TensorEngine.
The host unshard step sums the 8 partials and applies squash.

K-sharding moves ~14 MB/core from HBM vs ~151 MB/core for the data-parallel
(replicated-weight) alternative; the kernel runs at the bf16 TensorE roofline.
"""

import os
import sys

for _p in ("/opt/trn_rl_repo", "/root/.axon_site/_ro/trn_rl_repo"):
    if os.path.isdir(_p) and _p not in sys.path:
        sys.path.append(_p)

import ml_dtypes
import numpy as np

N_CAPS = 36
D = 1024
B = 512
N_CORES = 8
K = N_CAPS * D            # 36864 contraction length
KC = K // N_CORES         # 4608 per core
KT = KC // 128            # 36 k-tiles of 128 per core
B_TILES = B // 128        # 4
D_CHUNKS = D // 512       # 2

_CACHE = {}
LAST_RESULTS = None       # BassKernelResults of the most recent run (for profiling)


def _build():
    import concourse.mybir as mybir
    import concourse.tile as tile
    from concourse import bacc

    nc = bacc.Bacc("TRN2", target_bir_lowering=False, debug=False,
                   num_devices=N_CORES)

    # Per-core inputs, already transposed/tiled host-side:
    #   xt[p, kt*512 + b] = x[b, k0 + kt*128 + p]   (k = j*1024+e flattened)
    #   wt[p, kt*1024 + d] = W[0, j, d, e] at k = k0 + kt*128 + p
    xt = nc.dram_tensor("xt", [128, KT * B], mybir.dt.bfloat16, kind="ExternalInput")
    wt = nc.dram_tensor("wt", [128, KT * D], mybir.dt.bfloat16, kind="ExternalInput")
    out = nc.dram_tensor("out", [B, D], mybir.dt.float32, kind="ExternalOutput")

    with tile.TileContext(nc) as tc:
        with tc.tile_pool(name="xpool", bufs=1) as xpool, \
             tc.tile_pool(name="wpool", bufs=1) as wpool, \
             tc.tile_pool(name="stage", bufs=8) as stage_pool, \
             tc.tile_pool(name="psum", bufs=8, space="PSUM") as psum_pool:

            X = xpool.tile([128, KT * B], mybir.dt.bfloat16, name="X")
            W = wpool.tile([128, KT * D], mybir.dt.bfloat16, name="W")

            # Stream inputs in kt-chunks so matmuls can start on chunk 0
            # while later chunks are still in flight (subtile deps).
            CH = 6
            for c in range(KT // CH):
                s = c * CH
                nc.sync.dma_start(out=X[:, s * B:(s + CH) * B],
                                  in_=xt[:, s * B:(s + CH) * B])
                nc.sync.dma_start(out=W[:, s * D:(s + CH) * D],
                                  in_=wt[:, s * D:(s + CH) * D])

            # Two phases (d-chunk 0 then 1); 4 psum banks accumulate per
            # phase, so phase-0 PSUM->SBUF->DRAM drains overlap phase-1
            # matmuls and the kernel tail stays short.
            for d in range(D_CHUNKS):
                psums = []
                for b in range(B_TILES):
                    pt = psum_pool.tile([128, 512], mybir.dt.float32,
                                        name=f"ps_{d}_{b}", tag="ps")
                    psums.append(pt)
                for kt in range(KT):
                    for b in range(B_TILES):
                        nc.tensor.matmul(
                            psums[b][:, :],
                            lhsT=X[:, kt * B + b * 128: kt * B + (b + 1) * 128],
                            rhs=W[:, kt * D + d * 512: kt * D + (d + 1) * 512],
                            start=(kt == 0),
                            stop=(kt == KT - 1),
                        )
                for b in range(B_TILES):
                    st = stage_pool.tile([128, 512], mybir.dt.float32,
                                         name=f"st_{d}_{b}", tag="st")
                    nc.vector.tensor_copy(st[:, :], psums[b][:, :])
                    nc.sync.dma_start(
                        out=out[b * 128:(b + 1) * 128, d * 512:(d + 1) * 512],
                        in_=st[:, :])

    nc.compile()
    return nc


def _get_nc():
    if "nc" not in _CACHE:
        _CACHE["nc"] = _build()
    return _CACHE["nc"]


def _shard_inputs(x, weight):
    """Host-side layout prep: transpose to k-major, tile for SBUF, cast bf16."""
    bf16 = ml_dtypes.bfloat16
    # x: (B, 36, 1024) -> xT (K, B) -> per-core [128, KT*B]
    xT = np.ascontiguousarray(np.transpose(x, (1, 2, 0))).reshape(K, B)
    xts = (xT.reshape(N_CORES, KT, 128, B)
              .transpose(0, 2, 1, 3)
              .reshape(N_CORES, 128, KT * B)
              .astype(bf16))
    # weight: (1, 36, D, E) -> Wk (K, D) with k=(j,e) -> per-core [128, KT*D]
    wk = np.ascontiguousarray(np.transpose(weight[0], (0, 2, 1))).reshape(K, D)
    wts = (wk.reshape(N_CORES, KT, 128, D)
              .transpose(0, 2, 1, 3)
              .reshape(N_CORES, 128, KT * D)
              .astype(bf16))
    return xts, wts


def kernel(x, weight, isLastLayer=None):
    global LAST_RESULTS
    from concourse.bass_utils import run_bass_kernel_spmd

    x = np.asarray(x, dtype=np.float32)
    weight = np.asarray(weight, dtype=np.float32)
    assert x.shape == (B, N_CAPS, D) and weight.shape == (1, N_CAPS, D, D)

    xts, wts = _shard_inputs(x, weight)
    in_maps = [{"xt": np.ascontiguousarray(xts[i]),
                "wt": np.ascontiguousarray(wts[i])} for i in range(N_CORES)]

    nc = _get_nc()
    res = run_bass_kernel_spmd(nc, in_maps, core_ids=list(range(N_CORES)))
    LAST_RESULTS = res

    # Unshard: sum the 8 contraction partials, then squash.
    s = np.zeros((B, D), dtype=np.float32)
    for core_out in res.results:
        s += np.asarray(core_out["out"], dtype=np.float32)
    norm = np.sqrt((s.astype(np.float64) ** 2).sum(axis=-1, keepdims=True)).astype(np.float32)
    scale = norm ** 2 / (1.0 + norm ** 2) / (norm + 1e-8)
    return (scale * s)[:, None, :].astype(np.float32)
